# revision 27
# baseline (speedup 1.0000x reference)
"""Single-head attention (B=4, N=2048, D=1024) on 8 Trainium2 NeuronCores.

Sharding: core c handles batch c//2 and KEY half c%2.  Each core computes
K/V projections for its 1024 keys, Q for all 2048 queries of its batch, and
partial (unnormalized) attention output plus the partial softmax denominator
over its key half.  The host combines the two halves per batch:
out = (oA + oB) / (dA + dB).  This duplicates only the Q projection across a
core pair (the cheapest of the three), vs duplicating K and V.

All matmuls bf16 with fp32 PSUM accumulation; exp in fp32 on the scalar
engine.  Unnormalized softmax (no max subtraction) is safe: |scores/sqrt(D)|
is ~N(0, 0.33^2) for these inputs.
"""

from contextlib import ExitStack

import ml_dtypes
import numpy as np

import concourse.bass as bass
import concourse.mybir as mybir
import concourse.tile as tile
from concourse.bass_utils import run_bass_kernel_spmd

B, N, D = 4, 2048, 1024
NCORES = 8
P = 128
NQ = N            # queries per core (full batch)
NKH = N // 2      # keys per core (half)
DC = D // P       # 8 contraction chunks
EC = D // P       # 8 embed blocks
JB = NKH // P     # 8 key blocks
F = 512           # matmul moving free dim (one PSUM bank of fp32)
SCALE = 1.0 / np.sqrt(D)

BF = mybir.dt.bfloat16
F32 = mybir.dt.float32


def _attention_kernel(ctx, tc, out, xT, xTk, wqT, wkT, wvT):
    nc = tc.nc

    consts = ctx.enter_context(tc.tile_pool(name="consts", bufs=1))
    psmain = ctx.enter_context(tc.tile_pool(name="psmain", bufs=2, space="PSUM"))
    psav = ctx.enter_context(tc.tile_pool(name="psav", bufs=6, space="PSUM"))
    outp = ctx.enter_context(tc.tile_pool(name="outp", bufs=2))
    small = ctx.enter_context(tc.tile_pool(name="small", bufs=2))

    # Resident SBUF tensors (~170KB/partition).  qT shares the wk+wv buffer:
    # their last reads (phases 1a/1b) precede qT's first write (phase 2a).
    xT_sb = consts.tile([P, DC, NQ], BF, tag="xT")       # [p, d-chunk, query]
    xTk_sb = consts.tile([P, DC, NKH], BF, tag="xTk")    # [p, d-chunk, key]
    wkv_sb = consts.tile([P, 2 * DC * D], BF, tag="wkv")
    wk_sb = wkv_sb.rearrange("p (two c e) -> p two c e", two=2, c=DC)[:, 0]
    wv_sb = wkv_sb.rearrange("p (two c e) -> p two c e", two=2, c=DC)[:, 1]
    qT_sb = wkv_sb.rearrange("p (e i) -> p e i", e=EC)   # [p, e-block, query]
    wq_sb = consts.tile([P, DC, D], BF, tag="wq")
    kT_sb = consts.tile([P, EC, NKH], BF, tag="kT")      # [p, e-block, key]
    v_sb = consts.tile([P, JB, D], BF, tag="v")          # [p, key-block, e]
    pT_sb = consts.tile([P, JB, NQ], BF, tag="pT")       # [p, key-block, query]
    ones_sb = consts.tile([P, 1], BF, tag="ones")

    nc.vector.memset(ones_sb, 1.0)

    xTr = xT.rearrange("(c p) i -> p c i", p=P)
    xTkr = xTk.rearrange("(c p) j -> p c j", p=P)
    wqr = wqT.rearrange("(c p) e -> p c e", p=P)
    wkr = wkT.rearrange("(c p) e -> p c e", p=P)
    wvr = wvT.rearrange("(c p) e -> p c e", p=P)

    # Chunk-0 of every tensor first (the warm-up touch matmuls below gate
    # the PE on exactly these five chunks), then the rest in consumption
    # order (1a: wk+xTk; 1b: wv; 2a: wq+xT).
    in_dmas = []
    wkv_dmas = [None] * (2 * DC)
    wkv_dmas[0] = nc.sync.dma_start(out=wk_sb[:, 0, :], in_=wkr[:, 0, :])
    in_dmas.append(nc.sync.dma_start(out=xTk_sb[:, 0, :], in_=xTkr[:, 0, :]))
    wkv_dmas[DC] = nc.sync.dma_start(out=wv_sb[:, 0, :], in_=wvr[:, 0, :])
    in_dmas.append(nc.sync.dma_start(out=wq_sb[:, 0, :], in_=wqr[:, 0, :]))
    in_dmas.append(nc.sync.dma_start(out=xT_sb[:, 0, :], in_=xTr[:, 0, :]))
    for c in range(1, DC):
        wkv_dmas[c] = nc.sync.dma_start(out=wk_sb[:, c, :], in_=wkr[:, c, :])
        in_dmas.append(nc.sync.dma_start(out=xTk_sb[:, c, :], in_=xTkr[:, c, :]))
    for c in range(1, DC):
        wkv_dmas[DC + c] = nc.sync.dma_start(out=wv_sb[:, c, :], in_=wvr[:, c, :])
    for c in range(1, DC):
        in_dmas.append(nc.sync.dma_start(out=wq_sb[:, c, :], in_=wqr[:, c, :]))
        in_dmas.append(nc.sync.dma_start(out=xT_sb[:, c, :], in_=xTr[:, c, :]))
    in_dmas.extend(wkv_dmas)

    def sp_observe(inst, why):
        # One-wait nops on the sync sequencer: make SP observe a proc's sem
        # tick so later SP instructions (the kernel-tail drain) don't need
        # to aggregate multiple sync waits (HW allows one per instruction).
        n = nc.sync.nop(hint="observe")
        tile.add_dep_helper(n.ins, inst.ins, reason=why)

    # HAM pre-warm: keep the PE busy on dummy matmuls over zeroed SBUF while
    # the first input chunks are still in flight, so the PE clock-gate
    # (which needs ~3.4us of sustained activity) opens before real work.
    warm_src = small.tile([P, 640], BF, tag="warm")
    nc.vector.memset(warm_src, 0.0)
    warm_ps = psmain.tile([P, F], F32, tag="ps")
    N_WARM = 40
    for w in range(N_WARM):
        nc.tensor.matmul(
            warm_ps,
            lhsT=warm_src[:, 0:P],
            rhs=warm_src[:, P : P + F],
            start=(w == 0),
            stop=(w == N_WARM - 1),
        )

    # Warm-up touches: attach each input tensor's chunk-0 DMA wait to a
    # dedicated trivial matmul while PSUM slot reuse is still PE-local, so
    # no later matmul needs a DMA wait on top of a PSUM-WAR wait (PE
    # matmuls support a single sync-wait command).
    for t in (wk_sb, xTk_sb, wv_sb, wq_sb, xT_sb):
        wm = psmain.tile([P, F], F32, tag="ps")
        nc.tensor.matmul(
            wm[0:1, 0:1], lhsT=t[:, 0, 0:1], rhs=t[:, 0, 0:1], start=True, stop=True
        )

    # Phase 1a: kT[e, j] — lhsT = WkT[d, e-blk], rhs = xTk[d, j-tile]
    for e in range(EC):
        for jt in range(NKH // F):
            ps = psmain.tile([P, F], F32, tag="ps")
            for c in range(DC):
                nc.tensor.matmul(
                    ps,
                    lhsT=wk_sb[:, c, e * P : (e + 1) * P],
                    rhs=xTk_sb[:, c, jt * F : (jt + 1) * F],
                    start=(c == 0),
                    stop=(c == DC - 1),
                )
            nc.vector.tensor_copy(out=kT_sb[:, e, jt * F : (jt + 1) * F], in_=ps)

    # Phase 1b: v[j, e] — lhsT = xTk[d, j-blk], rhs = WvT[d, e-tile]
    for j in range(JB):
        for et in range(D // F):
            ps = psmain.tile([P, F], F32, tag="ps")
            for c in range(DC):
                nc.tensor.matmul(
                    ps,
                    lhsT=xTk_sb[:, c, j * P : (j + 1) * P],
                    rhs=wv_sb[:, c, et * F : (et + 1) * F],
                    start=(c == 0),
                    stop=(c == DC - 1),
                )
            nc.vector.tensor_copy(out=v_sb[:, j, et * F : (et + 1) * F], in_=ps)

    # DVE touches: qT overwrites the wk/wv buffer, so the DVE must have
    # observed those input DMAs before its first qT write (WAW), or the qT
    # copies would need a DMA wait on top of their PE wait.  Self-copies
    # carry the DMA waits via explicit deps only.
    touch = small.tile([P, 2 * DC], F32, tag="touch")
    for c in range(2 * DC):
        t = nc.vector.memset(touch[0:1, c : c + 1], 0.0)
        tile.add_dep_helper(t.ins, wkv_dmas[c].ins, reason="observe wkv DMA on DVE")

    # Phase 2a: qT[e, i] for ALL queries of the batch
    for e in range(EC):
        for it in range(NQ // F):
            ps = psmain.tile([P, F], F32, tag="ps")
            for c in range(DC):
                nc.tensor.matmul(
                    ps,
                    lhsT=wq_sb[:, c, e * P : (e + 1) * P],
                    rhs=xT_sb[:, c, it * F : (it + 1) * F],
                    start=(c == 0),
                    stop=(c == DC - 1),
                )
            nc.vector.tensor_copy(out=qT_sb[:, e, it * F : (it + 1) * F], in_=ps)

    # Phase 2b: scoresT[j, i] = k @ q.T over this key half, p = exp(s*SCALE)
    for j in range(JB):
        for it in range(NQ // F):
            ps = psmain.tile([P, F], F32, tag="ps")
            for e in range(EC):
                nc.tensor.matmul(
                    ps,
                    lhsT=kT_sb[:, e, j * P : (j + 1) * P],
                    rhs=qT_sb[:, e, it * F : (it + 1) * F],
                    start=(e == 0),
                    stop=(e == EC - 1),
                )
            last_exp = nc.scalar.activation(
                out=pT_sb[:, j, it * F : (it + 1) * F],
                in_=ps,
                func=mybir.ActivationFunctionType.Exp,
                scale=float(SCALE),
            )

    for d in in_dmas:
        sp_observe(d, "observe input DMA on SP")

    # Phase 2c: partial out[i, 0:1024] = pT.T @ v, partial denom in column
    # 1024 (folded into the same output tensor so there are exactly 8
    # stores — one lap of the 8 SWDGE queues; a second lap would add a
    # queue-order wait on top of the data-ready wait).
    outr = out.rearrange("(g two p) e -> g p two e", two=2, p=P)
    oguard = small.tile([P, NQ // (2 * P)], F32, tag="oguard")
    out_dmas = []
    for ib2 in range(NQ // (2 * P)):
        o_sb = outp.tile([P, 2, D + 1], F32, tag="o")
        g = None
        if ib2 >= 2:
            # Pre-observe the output-DMA tick (WAR on o_sb slot reuse) on
            # the DVE so the copies below carry only their one data wait.
            g = nc.vector.memset(oguard[0:1, ib2 : ib2 + 1], 0.0)
            tile.add_dep_helper(
                g.ins, out_dmas[ib2 - 2].ins, reason="observe out DMA on DVE"
            )
        # Absorb the WAW against the slot's previous DVE writes in a guard
        # write of its own, so the data copies keep a single wait each.
        g2 = nc.vector.memset(o_sb[0:1, 0, 0:1], 0.0)
        if g is not None:
            tile.add_dep_helper(g2.ins, g.ins, False, reason="order after oguard")
        for t in range(2):
            ib = 2 * ib2 + t
            po0 = psav.tile([P, F], F32, tag="po")
            po1 = psav.tile([P, F], F32, tag="po")
            pd = psav.tile([P, F], F32, tag="po")
            for j in range(JB):
                lhsT = pT_sb[:, j, ib * P : (ib + 1) * P]
                nc.tensor.matmul(
                    po0, lhsT=lhsT, rhs=v_sb[:, j, 0:F],
                    start=(j == 0), stop=(j == JB - 1),
                )
                nc.tensor.matmul(
                    po1, lhsT=lhsT, rhs=v_sb[:, j, F : 2 * F],
                    start=(j == 0), stop=(j == JB - 1),
                )
                last_mm = nc.tensor.matmul(
                    pd[:, 0:1], lhsT=lhsT, rhs=ones_sb,
                    start=(j == 0), stop=(j == JB - 1),
                )
            # Denominator copy first: pd's stop-matmul is the group's last
            # PE tick, so this copy's PE wait covers po0/po1 and the po
            # copies need only their (buffer-reuse) DVE wait.  The explicit
            # sync=False deps pin the scheduler to that order.
            dcp = nc.vector.tensor_copy(
                out=o_sb[:, t, D : D + 1], in_=pd[:, 0:1]
            )
            tile.add_dep_helper(dcp.ins, g2.ins, False, reason="order after guard")
            c0 = nc.vector.tensor_copy(out=o_sb[:, t, 0:F], in_=po0)
            tile.add_dep_helper(c0.ins, dcp.ins, False, reason="order after dcp")
            last_cp = nc.vector.tensor_copy(out=o_sb[:, t, F : 2 * F], in_=po1)
            tile.add_dep_helper(last_cp.ins, c0.ins, False, reason="order after c0")
        out_dmas.append(nc.gpsimd.dma_start(out=outr[ib2], in_=o_sb))

    # Let SP observe every remaining proc's final tick so the auto-generated
    # kernel-tail drain needs no aggregated multi-sem wait of its own.
    for dd in out_dmas:
        sp_observe(dd, "observe output DMA on SP")
    sp_observe(last_exp, "observe ACT on SP")
    sp_observe(last_mm, "observe PE on SP")
    sp_observe(last_cp, "observe DVE on SP")


def build_attention_module():
    nc = bass.Bass(trn_type="TRN2", target_bir_lowering=False, debug=False)
    xT = nc.dram_tensor("xT", [D, NQ], BF, kind="ExternalInput").ap()
    xTk = nc.dram_tensor("xTk", [D, NKH], BF, kind="ExternalInput").ap()
    wqT = nc.dram_tensor("wqT", [D, D], BF, kind="ExternalInput").ap()
    wkT = nc.dram_tensor("wkT", [D, D], BF, kind="ExternalInput").ap()
    wvT = nc.dram_tensor("wvT", [D, D], BF, kind="ExternalInput").ap()
    out = nc.dram_tensor("out", [NQ, D + 1], F32, kind="ExternalOutput").ap()
    with tile.TileContext(nc) as tc:
        with ExitStack() as ctx:
            _attention_kernel(ctx, tc, out, xT, xTk, wqT, wkT, wvT)
    return nc


_module_cache = None


def _get_module():
    global _module_cache
    if _module_cache is None:
        _module_cache = build_attention_module()
    return _module_cache


def make_in_maps(x, Wq, Wk, Wv):
    bf = ml_dtypes.bfloat16
    x = np.asarray(x, dtype=np.float32)
    wq = np.asarray(Wq, dtype=np.float32).T.astype(bf)
    wk = np.asarray(Wk, dtype=np.float32).T.astype(bf)
    wv = np.asarray(Wv, dtype=np.float32).T.astype(bf)
    in_maps = []
    for core in range(NCORES):
        b, half = divmod(core, 2)
        xt = np.ascontiguousarray(x[b].T).astype(bf)  # [D, N]
        in_maps.append(
            {
                "xT": xt,
                "xTk": np.ascontiguousarray(xt[:, half * NKH : (half + 1) * NKH]),
                "wqT": wq,
                "wkT": wk,
                "wvT": wv,
            }
        )
    return in_maps


def _install_ntff_hook_shim():
    """The container's `antenv` stub lacks axon_hooks; register an equivalent
    built on trn_agent_boot's ctypes NTFF driver so trace=True works."""
    import sys
    import types

    if "antenv.axon_hooks" in sys.modules:
        return
    try:
        from trn_agent_boot.trn_boot import _ntff_profile_via_ctypes

        hook = _ntff_profile_via_ctypes("/opt/axon/libaxon_pjrt.so")
    except Exception:
        hook = None
    mod = types.ModuleType("antenv.axon_hooks")
    mod.get_axon_ntff_profile_hook = lambda: hook
    sys.modules["antenv.axon_hooks"] = mod


def kernel(x, Wq, Wk, Wv, _trace=False, _trace_cores=None):
    if _trace:
        _install_ntff_hook_shim()
    in_maps = make_in_maps(x, Wq, Wk, Wv)
    nc = _get_module()
    res = run_bass_kernel_spmd(
        nc,
        in_maps,
        core_ids=list(range(NCORES)),
        trace=_trace,
        trace_cores=_trace_cores,
    )
    out = np.empty((B, N, D), dtype=np.float32)
    for b in range(B):
        r0, r1 = res.results[2 * b], res.results[2 * b + 1]
        osum = r0["out"] + r1["out"]
        out[b] = osum[:, :D] / osum[:, D : D + 1]
    if _trace:
        return out, res
    return out
